# revision 33
# baseline (speedup 1.0000x reference)
"""MetaRoPE kernel for Trainium2, 8 NeuronCores — fp16 I/O + DVE 2x-mode.

Reference computation:
    r = rotate_m[token_positions]            # [S, D, D], block-diag 2x2 rotations
    out = einsum('bhsi,soi->bhso', x, r)     # x: [4, 32, 4096, 64] fp32

Because r is block-diagonal with 2x2 blocks, for each position s and pair k:
    out[2k]   = a*x[2k] + b*x[2k+1]     (a = r[2k,2k],   b = r[2k,2k+1])
    out[2k+1] = c*x[2k+1] + d*x[2k]     (c = r[2k+1,2k+1], d = r[2k+1,2k])
which we compute elementwise as
    out = x * A + pairswap(x * B')
with host-precomputed tables A, B' of shape [S, D]:
    A[s,2k] = a, A[s,2k+1] = c
    B'[s,2k] = d, B'[s,2k+1] = b       (B' is pre-pairswapped so that
                                        pairswap(x*B') lands b*x_odd on even
                                        lanes and d*x_even on odd lanes)

Precision/bandwidth: the correctness gate is rel_err < 2e-2; fp16 end-to-end
(host converts x fp32->fp16, device computes in fp16, host converts the fp16
result back) measures ~1.1e-3 and halves both HBM traffic and DVE element
cost vs fp32. Plain InstTensorTensor ops hit the DVE 2x_1p perf mode with
packed fp16 (~0.5 ns/elem/partition measured, including the stride -1
pair-swap operand). Notes from measurement on HW:
  - scalar_tensor_tensor (fused 3-input op) supports NO DVE perf modes and
    runs ~1.2 ns/elem — slower than two plain ops.
  - 4-dim merged APs (one mul writing u and o via a broadcast x) run ~15%
    slower per element than 3-dim APs.
  - GpSimd tensor ops are Q7 software (~2.5-6.4 ns/elem) AND degrade
    concurrent DVE throughput — never offload to it. Re-verified: a
    single-slab GpSimd sidecar (quarter pieces, interleaved stores)
    measured 81.9 us (+10.6): GpSimd is also the framework's semaphore
    relay hub, so tensor ops in its queue stall every other engine.
  - TENSOR_TENSOR with an int8 output drops to 1x mode (+17 us measured):
    int8 output needs a separate cast, which costs more than it saves here.
  - Stores measure only ~142 GB/s/core (loads ~314 GB/s/core) regardless
    of ring count, burst size or layout; loads and stores overlap fully
    (duplex), but loads split across BOTH rings still cap ~320 GB/s total.
    GpSimd SWDGE stores hit the same wall (~130 GB/s, desc-gen overhead)
    and scalar+gpsimd mixed stores do NOT combine — the write cap is
    global across all three DMA paths.
  - int8 stores with the cast on ScalarE (tables pre-scaled 127/8):
    77.0 us; with the last chunk's adds writing int8 directly (1x) to
    skip the final cast hop: 76.1 us. Root cause int8 cannot win: at
    the 36-op plan the fp16 store drain (ends ~68.5) already finishes
    inside the add-stream shadow (~68-69) — the store wall is NOT
    binding, so halving store bytes saves ~1 us of final drain while
    any cast path costs at least that in added stream/queue latency.
  - Chunk-plan space is closed from both ends: early chunks must be
    small for LOAD pacing ([1,2,4,4,4,1] = 88.4 us, 4-slab chunks
    before ~row 8 starve) and late chunks small for STORE pacing
    ([1,1,2,4,6,1,1] = 76.7 us — a 6-slab chunk's 3 MB store cannot
    issue until its single add completes, pushing the drain past the
    stream end). [1,1,2,4,4,2,1,1] is the optimum.
  - The ~2 us early TT gap (second compute piece waiting second table
    halves) is load-bandwidth-conserved: 1.5 MB of tables+slab0 must
    land before second-half compute vs ~0.7 us of available compute —
    no reorder removes it; only smaller tables would, and fp8 tables
    fail the accuracy gate.
  - Chunk plans feeding 4-slab chunks before ~row 8 starve the DVE (the
    load stream delivers ~1 slab/1.64 us after tables): [1,2,4,4,4,1]
    measured 88.4 us. The shipped plan keeps 1/1/2-slab chunks up front.
  - Run-to-run variance for the same NEFF is +-1.5-2 us (72.07-74.26
    observed), from engine-init and DMA-latency jitter.
  - The idle TensorE cannot profitably offload elementwise work: matmul
    output is PSUM-fp32-only on TRN2, and evacuating PSUM costs ~1.3
    ns/elem (ScalarE flat ACTIVATE copy; 4.9 ns/elem if the copy AP is
    strided) per pass — more than the ~1.6 ns/elem the DVE 3-pass spends
    outright. A full two-round block-diag-matmul hybrid (R_s = R_{64q} R_r,
    96 weight tiles, 14 DVE + 2 PE slabs) was built and measured 75-119 us
    across schedules: the extra ~4 MB of weight/relayout loads starves the
    DVE mid-stream at the shared load ceiling (see kernel_hybrid_pe.py.bak).

Sharding: x reshaped to [128 (b,h) slabs, 4096, 64]; 16 slabs per core.
Each slab [4096*64] is viewed as [128 partitions, 2048 free] (contiguous per
partition; partition p holds positions 32p..32p+31). Tables are replicated
to every core as [128, 2048] fp16 tiles that match that layout for every
slab.

Per core the 16 slabs are processed in chunks (CHUNK_PLAN, tapered small at
the ends to shrink pipeline ramp/tail; 4-slab middle chunks cut the DVE
instruction count ~42 -> ~30, saving ~1.6 us vs the all-2-slab plan). Each
chunk: one load (HWDGE on the sync ring), two DVE tensor_muls (tables
broadcast across the chunk's slabs via a step-0 AP dim) + one pair-swapped
in-place tensor_add, one store (HWDGE on the scalar ring). Table halves
split across rings: first halves on the scalar ring up front, second halves
on the sync ring emitted after the head chunk's first half-compute, so that
compute (which needs only table cols [0:1024)) starts ~3.5 us earlier —
tile deps follow emission order, and every DVE op emitted after a DMA to
tb/ta waits on it.

Measured: 71.2-72.5 us HW exec across runs (149.8 us fp32 naive ->
74.3 us all-2-slab plan -> this). Closed-form breakdown: first TT at
~10.8-11.4 (engine init ~7 + table/first-chunk DMA), one conserved
~2 us gap (second table halves), 54.4 us TT stream (51.2 roofline +
~3 instr overhead, otherwise gapless), last store ~1.5, end barrier
~2.7. Stores drain continuously from ~9.5 and finish inside the
stream shadow. Every component is individually floored: init/barrier
are framework-fixed, the stream is at the 7-lane-crossbar silicon
limit for fp16 TT, the early gap is load-BW-conserved, and the
int8 / TensorE / GpSimd alternatives all measure worse (notes above).
"""

import sys

import numpy as np

_TRN_REPO = "/opt/trn_rl_repo"
if _TRN_REPO not in sys.path:
    sys.path.insert(0, _TRN_REPO)

B, H, S, D = 4, 32, 4096, 64
BH = B * H                      # 128 (b,h) slabs
N_CORES = 8
BH_PER_CORE = BH // N_CORES     # 16 slabs per core
FREE = (S // 128) * D           # 2048 free elements per partition per slab
ROWS = BH_PER_CORE * 128        # 2048 dram rows per core, [ROWS, FREE] fp16
# slabs per chunk, tapered: small first chunks so compute starts early,
# small last chunk so the final store is short
CHUNK_PLAN = [1, 1, 2, 4, 4, 2, 1, 1]
assert sum(CHUNK_PLAN) == BH_PER_CORE
U_BUFS = 2

_prog_cache = {}


def _build_program():
    """Build (and cache) the SPMD Bass program for one core."""
    if "nc" in _prog_cache:
        return _prog_cache["nc"]

    import concourse.bacc as bacc
    import concourse.bass as bass
    import concourse.mybir as mybir
    import concourse.tile as tile

    f16 = mybir.dt.float16
    nc = bacc.Bacc(
        "TRN2", target_bir_lowering=False, debug=False, num_devices=N_CORES
    )
    x_d = nc.dram_tensor("x", [ROWS, FREE], f16, kind="ExternalInput").ap()
    ta_d = nc.dram_tensor("ta", [128, FREE], f16, kind="ExternalInput").ap()
    tb_d = nc.dram_tensor("tb", [128, FREE], f16, kind="ExternalInput").ap()
    o_d = nc.dram_tensor("out", [ROWS, FREE], f16, kind="ExternalOutput").ap()

    with tile.TileContext(nc) as tc:
        with (
            tc.tile_pool(name="tabs", bufs=1) as tabs,
            tc.tile_pool(name="xbig", bufs=1) as xbigp,
            tc.tile_pool(name="u", bufs=U_BUFS) as upool,
            tc.tile_pool(name="obig", bufs=1) as obigp,
        ):
            # table loads go on the scalar HWDGE ring (idle at start) so
            # they overlap the first x-chunk load on the sync ring; halves
            # ordered so the first half-slab compute (needs tb+ta cols
            # [0:hf)) can start before the full tables land
            tb = tabs.tile([128, FREE], f16)
            ta = tabs.tile([128, FREE], f16)
            hf = FREE // 2
            # NOTE: only the SP (sync) and Activation (scalar) rings are
            # HWDGE-capable here — a third queue via nc.vector.dma_start is
            # rejected by bass, and gpsimd SWDGE pays software desc-gen.
            # So the two first halves serialize on the scalar ring; quarter
            # pieces and ring-splitting were both tried and did not move
            # the first multiply earlier.
            nc.scalar.dma_start(tb[:, :hf], tb_d[:, :hf])
            nc.scalar.dma_start(ta[:, :hf], ta_d[:, :hf])
            # second halves are loaded from inside the chunk loop (on the
            # sync ring, after the first x chunk) so the head chunk's first
            # half-compute — emitted before them — only depends on the
            # first-half table loads (tile deps follow emission order).
            # (Tried instead putting ta's first half on the sync ring ahead
            # of x: per-queue transfer serialization pushed the first x half
            # later and the first multiply slipped ~2 us — keep both first
            # halves on the scalar ring.)

            # x and out live in single whole-core SBUF buffers (64 KiB per
            # partition each): every slice is written once and read once, so
            # there are no tile-reuse WAR waits — the only semaphores left
            # are load-done -> mul and add-done -> store per chunk
            xbig = xbigp.tile([128, BH_PER_CORE * FREE], f16)
            obig = obigp.tile([128, BH_PER_CORE * FREE], f16)

            def compute(xoff, ut, nsl, lo, sz):
                """u = x*tb; o = x*ta; o += pairswap(u) on cols [lo, lo+sz)
                of each of the nsl slabs at element offset xoff in the big
                x/out buffers (3-dim APs throughout)."""
                if nsl == 1:
                    xs = xbig[:, xoff + lo : xoff + lo + sz]
                    us = ut[:, lo : lo + sz]
                    os_ = obig[:, xoff + lo : xoff + lo + sz]
                    nc.vector.tensor_mul(us, xs, tb[:, lo : lo + sz])
                    nc.vector.tensor_mul(os_, xs, ta[:, lo : lo + sz])
                else:
                    assert lo == 0 and sz == nsl * FREE
                    cf = nsl * FREE
                    x3 = xbig[:, xoff : xoff + cf].rearrange(
                        "p (j f) -> p j f", j=nsl
                    )
                    u3 = ut[:].rearrange("p (j f) -> p j f", j=nsl)
                    os_ = obig[:, xoff : xoff + cf]
                    o3 = os_.rearrange("p (j f) -> p j f", j=nsl)
                    ta_b = bass.AP(
                        ta[:].tensor, ta[:].offset,
                        [ta[:].ap[0], [0, nsl], ta[:].ap[1]],
                    )
                    tb_b = bass.AP(
                        tb[:].tensor, tb[:].offset,
                        [tb[:].ap[0], [0, nsl], tb[:].ap[1]],
                    )
                    nc.vector.tensor_mul(u3, x3, tb_b)
                    nc.vector.tensor_mul(o3, x3, ta_b)
                    us = ut[:]
                usw = us.rearrange("p (n two) -> p n two", two=2)[:, :, ::-1]
                os3 = os_.rearrange("p (n two) -> p n two", two=2)
                nc.vector.tensor_add(os3, os3, usw)

            row0 = 0
            for ci, nsl in enumerate(CHUNK_PLAN):
                first = ci == 0
                last = ci == len(CHUNK_PLAN) - 1
                cfree = nsl * FREE
                xoff = row0 * FREE
                rows = x_d[row0 * 128 : (row0 + nsl) * 128, :]
                xts = xbig[:, xoff : xoff + cfree]
                if first:
                    # split the first load so compute can start after 0.25 MiB
                    assert nsl == 1
                    h = cfree // 2
                    nc.sync.dma_start(xts[:, :h], rows[:, :h])
                    nc.sync.dma_start(xts[:, h:], rows[:, h:])
                else:
                    src = rows.rearrange("(j p) f -> p j f", j=nsl)
                    nc.sync.dma_start(
                        xts.rearrange("p (j f) -> p j f", j=nsl), src
                    )

                ut = upool.tile([128, cfree], f16, tag="u")
                ots = obig[:, xoff : xoff + cfree]
                orows = o_d[row0 * 128 : (row0 + nsl) * 128, :]

                if first or last:
                    # head chunk in halves: starts computing after the first
                    # half-load (quartering the head measured 72.6 us vs
                    # 72.1 — the first add is table-DMA-gated, not
                    # compute-gated, so smaller pieces don't help). Tail
                    # chunk in quarters: the final store is only 0.125 MB,
                    # shrinking the end-of-run barrier wait for it.
                    nparts = 2 if first else 4
                    h = cfree // nparts
                    for hi in range(nparts):
                        compute(xoff, ut, 1, hi * h, h)
                        if first and hi == 0:
                            # second table halves, after the head's first
                            # half-compute in emission order. On the SCALAR
                            # ring: it goes idle at ~9.8us (after the first
                            # halves) while sync still drains x0 — t2 lands
                            # ~11.4 instead of ~13.5 queued behind x, and
                            # sync keeps feeding x chunks undisturbed.
                            nc.scalar.dma_start(tb[:, hf:], tb_d[:, hf:])
                            nc.scalar.dma_start(ta[:, hf:], ta_d[:, hf:])
                        nc.scalar.dma_start(
                            orows[:, hi * h : (hi + 1) * h],
                            ots[:, hi * h : (hi + 1) * h],
                        )
                else:
                    compute(xoff, ut, nsl, 0, cfree)
                    dst = orows.rearrange("(j p) f -> p j f", j=nsl)
                    nc.scalar.dma_start(
                        dst, ots.rearrange("p (j f) -> p j f", j=nsl)
                    )
                row0 += nsl

    nc.compile()
    _prog_cache["nc"] = nc
    return nc


def _default_rotate_m(theta=10000.0):
    """Rebuild the reference's rotation buffer if the harness doesn't pass it."""
    half = D // 2
    try:  # replicate the reference's jax-f32 arithmetic exactly if possible
        import jax.numpy as jnp

        pos = np.asarray(jnp.arange(S, dtype=jnp.float32))
        inv_freq = np.asarray(
            theta ** (-(2.0 * jnp.arange(half, dtype=jnp.float32)) / D)
        )
        ang = np.asarray(pos[:, None] * inv_freq[None, :], dtype=np.float32)
        c, s = np.asarray(jnp.cos(ang)), np.asarray(jnp.sin(ang))
    except Exception:
        pos = np.arange(S, dtype=np.float32)
        exp = (-(2.0 * np.arange(half, dtype=np.float32)) / D).astype(np.float32)
        inv_freq = np.power(np.float32(theta), exp, dtype=np.float32)
        ang = (pos[:, None] * inv_freq[None, :]).astype(np.float32)
        c, s = np.cos(ang, dtype=np.float32), np.sin(ang, dtype=np.float32)
    idx = 2 * np.arange(half)
    r = np.zeros((S, D, D), dtype=np.float32)
    r[:, idx, idx] = c
    r[:, idx, idx + 1] = -s
    r[:, idx + 1, idx] = s
    r[:, idx + 1, idx + 1] = c
    return r


def _tables(token_positions, rotate_m):
    """Host-precompute the [128, FREE] fp16 A and B' tables (see docstring)."""
    if rotate_m is None:
        rotate_m = _default_rotate_m()
    r = np.asarray(rotate_m, dtype=np.float32)[np.asarray(token_positions)]
    idx = np.arange(D // 2) * 2
    a = r[:, idx, idx]            # x_even -> out_even
    b = r[:, idx, idx + 1]        # x_odd  -> out_even
    c = r[:, idx + 1, idx + 1]    # x_odd  -> out_odd
    d = r[:, idx + 1, idx]        # x_even -> out_odd
    A = np.empty((S, D), np.float32)
    A[:, 0::2] = a
    A[:, 1::2] = c
    Bp = np.empty((S, D), np.float32)
    Bp[:, 0::2] = d
    Bp[:, 1::2] = b
    return (
        np.ascontiguousarray(A.reshape(128, FREE)).astype(np.float16),
        np.ascontiguousarray(Bp.reshape(128, FREE)).astype(np.float16),
    )


def _in_maps(x, token_positions, rotate_m):
    ta, tb = _tables(token_positions, rotate_m)
    xs = np.asarray(x, dtype=np.float32).astype(np.float16).reshape(
        N_CORES, ROWS, FREE
    )
    xs = np.ascontiguousarray(xs)
    return [{"x": xs[i], "ta": ta, "tb": tb} for i in range(N_CORES)]


def _run(x, token_positions, rotate_m=None, trace=False, trace_cores=None):
    from concourse.bass_utils import run_bass_kernel_spmd

    nc = _build_program()
    in_maps = _in_maps(x, token_positions, rotate_m)
    res = run_bass_kernel_spmd(
        nc,
        in_maps,
        list(range(N_CORES)),
        trace=trace,
        trace_cores=trace_cores,
    )
    out = np.concatenate(
        [res.results[i]["out"].reshape(1, ROWS * FREE) for i in range(N_CORES)]
    ).reshape(B, H, S, D).astype(np.float32)
    return out, res


def kernel(x, token_positions, rotate_m=None, **_unused):
    out, _ = _run(x, token_positions, rotate_m, trace=False)
    return out


# revision 34
# speedup vs baseline: 1.0552x; 1.0552x over previous
"""MetaRoPE kernel for Trainium2, 8 NeuronCores — fp16 I/O + DVE 2x-mode.

Reference computation:
    r = rotate_m[token_positions]            # [S, D, D], block-diag 2x2 rotations
    out = einsum('bhsi,soi->bhso', x, r)     # x: [4, 32, 4096, 64] fp32

Because r is block-diagonal with 2x2 blocks, for each position s and pair k:
    out[2k]   = a*x[2k] + b*x[2k+1]     (a = r[2k,2k],   b = r[2k,2k+1])
    out[2k+1] = c*x[2k+1] + d*x[2k]     (c = r[2k+1,2k+1], d = r[2k+1,2k])
which we compute elementwise as
    out = x * A + pairswap(x * B')
with host-precomputed tables A, B' of shape [S, D]:
    A[s,2k] = a, A[s,2k+1] = c
    B'[s,2k] = d, B'[s,2k+1] = b       (B' is pre-pairswapped so that
                                        pairswap(x*B') lands b*x_odd on even
                                        lanes and d*x_even on odd lanes)

Precision/bandwidth: the correctness gate is rel_err < 2e-2; fp16 end-to-end
(host converts x fp32->fp16, device computes in fp16, host converts the fp16
result back) measures ~1.1e-3 and halves both HBM traffic and DVE element
cost vs fp32. Plain InstTensorTensor ops hit the DVE 2x_1p perf mode with
packed fp16 (~0.5 ns/elem/partition measured, including the stride -1
pair-swap operand). Notes from measurement on HW:
  - scalar_tensor_tensor (fused 3-input op) supports NO DVE perf modes and
    runs ~1.2 ns/elem — slower than two plain ops.
  - 4-dim merged APs (one mul writing u and o via a broadcast x) run ~15%
    slower per element than 3-dim APs.
  - GpSimd tensor ops are Q7 software (~2.5-6.4 ns/elem) AND degrade
    concurrent DVE throughput — never offload to it. Re-verified: a
    single-slab GpSimd sidecar (quarter pieces, interleaved stores)
    measured 81.9 us (+10.6): GpSimd is also the framework's semaphore
    relay hub, so tensor ops in its queue stall every other engine.
  - TENSOR_TENSOR with an int8 output drops to 1x mode (+17 us measured):
    int8 output needs a separate cast, which costs more than it saves here.
  - Stores measure only ~142 GB/s/core (loads ~314 GB/s/core) regardless
    of ring count, burst size or layout; loads and stores overlap fully
    (duplex), but loads split across BOTH rings still cap ~320 GB/s total.
    GpSimd SWDGE stores hit the same wall (~130 GB/s, desc-gen overhead)
    and scalar+gpsimd mixed stores do NOT combine — the write cap is
    global across all three DMA paths.
  - int8 stores with the cast on ScalarE (tables pre-scaled 127/8):
    77.0 us; with the last chunk's adds writing int8 directly (1x) to
    skip the final cast hop: 76.1 us. Root cause int8 cannot win: at
    the 36-op plan the fp16 store drain (ends ~68.5) already finishes
    inside the add-stream shadow (~68-69) — the store wall is NOT
    binding, so halving store bytes saves ~1 us of final drain while
    any cast path costs at least that in added stream/queue latency.
  - Chunk-plan space is closed from both ends: early chunks must be
    small for LOAD pacing ([1,2,4,4,4,1] = 88.4 us, 4-slab chunks
    before ~row 8 starve) and late chunks small for STORE pacing
    ([1,1,2,4,6,1,1] = 76.7 us — a 6-slab chunk's 3 MB store cannot
    issue until its single add completes, pushing the drain past the
    stream end). [1,1,2,4,4,2,1,1] is the optimum.
  - The ~2 us early TT gap (second compute piece waiting second table
    halves) is load-bandwidth-conserved: 1.5 MB of tables+slab0 must
    land before second-half compute vs ~0.7 us of available compute —
    no reorder removes it; only smaller tables would, and fp8 tables
    fail the accuracy gate.
  - Chunk plans feeding 4-slab chunks before ~row 8 starve the DVE (the
    load stream delivers ~1 slab/1.64 us after tables): [1,2,4,4,4,1]
    measured 88.4 us. The shipped plan keeps 1/1/2-slab chunks up front.
  - Run-to-run variance for the same NEFF is +-1.5-2 us (72.07-74.26
    observed), from engine-init and DMA-latency jitter.
  - The idle TensorE cannot profitably offload elementwise work: matmul
    output is PSUM-fp32-only on TRN2, and evacuating PSUM costs ~1.3
    ns/elem (ScalarE flat ACTIVATE copy; 4.9 ns/elem if the copy AP is
    strided) per pass — more than the ~1.6 ns/elem the DVE 3-pass spends
    outright. A full two-round block-diag-matmul hybrid (R_s = R_{64q} R_r,
    96 weight tiles, 14 DVE + 2 PE slabs) was built and measured 75-119 us
    across schedules: the extra ~4 MB of weight/relayout loads starves the
    DVE mid-stream at the shared load ceiling (see kernel_hybrid_pe.py.bak).

Sharding: x reshaped to [128 (b,h) slabs, 4096, 64]; 16 slabs per core.
Each slab [4096*64] is viewed as [128 partitions, 2048 free] (contiguous per
partition; partition p holds positions 32p..32p+31). Tables are replicated
to every core as [128, 2048] fp16 tiles that match that layout for every
slab.

Per core the 16 slabs are processed in chunks (CHUNK_PLAN, tapered small at
the ends to shrink pipeline ramp/tail; 4-slab middle chunks cut the DVE
instruction count ~42 -> ~30, saving ~1.6 us vs the all-2-slab plan). Each
chunk: one load (HWDGE on the sync ring), two DVE tensor_muls (tables
broadcast across the chunk's slabs via a step-0 AP dim) + one pair-swapped
in-place tensor_add, one store (HWDGE on the scalar ring). Table halves
split across rings: first halves on the scalar ring up front, second halves
on the sync ring emitted after the head chunk's first half-compute, so that
compute (which needs only table cols [0:1024)) starts ~3.5 us earlier —
tile deps follow emission order, and every DVE op emitted after a DMA to
tb/ta waits on it.

Measured: 71.2-72.5 us HW exec across runs (149.8 us fp32 naive ->
74.3 us all-2-slab plan -> this). Closed-form breakdown: first TT at
~10.8-11.4 (engine init ~7 + table/first-chunk DMA), one conserved
~2 us gap (second table halves), 54.4 us TT stream (51.2 roofline +
~3 instr overhead, otherwise gapless), last store ~1.5, end barrier
~2.7. Stores drain continuously from ~9.5 and finish inside the
stream shadow. Every component is individually floored: init/barrier
are framework-fixed, the stream is at the 7-lane-crossbar silicon
limit for fp16 TT, the early gap is load-BW-conserved, and the
int8 / TensorE / GpSimd alternatives all measure worse (notes above).
"""

import sys

import numpy as np

_TRN_REPO = "/opt/trn_rl_repo"
if _TRN_REPO not in sys.path:
    sys.path.insert(0, _TRN_REPO)

B, H, S, D = 4, 32, 4096, 64
BH = B * H                      # 128 (b,h) slabs
N_CORES = 8
BH_PER_CORE = BH // N_CORES     # 16 slabs per core
FREE = (S // 128) * D           # 2048 free elements per partition per slab
ROWS = BH_PER_CORE * 128        # 2048 dram rows per core, [ROWS, FREE] fp16
# slabs per chunk, tapered: small first chunks so compute starts early,
# small last chunk so the final store is short
CHUNK_PLAN = [1, 1, 2, 4, 4, 2, 1, 1]
assert sum(CHUNK_PLAN) == BH_PER_CORE
U_BUFS = 2

_prog_cache = {}


def _build_program():
    """Build (and cache) the SPMD Bass program for one core."""
    if "nc" in _prog_cache:
        return _prog_cache["nc"]

    import concourse.bacc as bacc
    import concourse.bass as bass
    import concourse.mybir as mybir
    import concourse.tile as tile

    f16 = mybir.dt.float16
    nc = bacc.Bacc(
        "TRN2", target_bir_lowering=False, debug=False, num_devices=N_CORES
    )
    x_d = nc.dram_tensor("x", [ROWS, FREE], f16, kind="ExternalInput").ap()
    ta_d = nc.dram_tensor("ta", [128, FREE], f16, kind="ExternalInput").ap()
    tb_d = nc.dram_tensor("tb", [128, FREE], f16, kind="ExternalInput").ap()
    o_d = nc.dram_tensor("out", [ROWS, FREE], f16, kind="ExternalOutput").ap()

    with tile.TileContext(nc) as tc:
        with (
            tc.tile_pool(name="tabs", bufs=1) as tabs,
            tc.tile_pool(name="xbig", bufs=1) as xbigp,
            tc.tile_pool(name="u", bufs=U_BUFS) as upool,
            tc.tile_pool(name="obig", bufs=1) as obigp,
        ):
            # table loads go on the scalar HWDGE ring (idle at start) so
            # they overlap the first x-chunk load on the sync ring; halves
            # ordered so the first half-slab compute (needs tb+ta cols
            # [0:hf)) can start before the full tables land
            tb = tabs.tile([128, FREE], f16)
            ta = tabs.tile([128, FREE], f16)
            hf = FREE // 2
            # NOTE: only the SP (sync) and Activation (scalar) rings are
            # HWDGE-capable here — a third queue via nc.vector.dma_start is
            # rejected by bass, and gpsimd SWDGE pays software desc-gen.
            # So the two first halves serialize on the scalar ring; quarter
            # pieces and ring-splitting were both tried and did not move
            # the first multiply earlier.
            nc.scalar.dma_start(tb[:, :hf], tb_d[:, :hf])
            nc.scalar.dma_start(ta[:, :hf], ta_d[:, :hf])
            # second halves are loaded from inside the chunk loop (on the
            # sync ring, after the first x chunk) so the head chunk's first
            # half-compute — emitted before them — only depends on the
            # first-half table loads (tile deps follow emission order).
            # (Tried instead putting ta's first half on the sync ring ahead
            # of x: per-queue transfer serialization pushed the first x half
            # later and the first multiply slipped ~2 us — keep both first
            # halves on the scalar ring.)

            # x and out live in single whole-core SBUF buffers (64 KiB per
            # partition each): every slice is written once and read once, so
            # there are no tile-reuse WAR waits — the only semaphores left
            # are load-done -> mul and add-done -> store per chunk
            xbig = xbigp.tile([128, BH_PER_CORE * FREE], f16)
            obig = obigp.tile([128, BH_PER_CORE * FREE], f16)

            def compute(xoff, ut, nsl, lo, sz):
                """u = x*tb; o = x*ta; o += pairswap(u) on cols [lo, lo+sz)
                of each of the nsl slabs at element offset xoff in the big
                x/out buffers (3-dim APs throughout)."""
                if nsl == 1:
                    xs = xbig[:, xoff + lo : xoff + lo + sz]
                    us = ut[:, lo : lo + sz]
                    os_ = obig[:, xoff + lo : xoff + lo + sz]
                    nc.vector.tensor_mul(us, xs, tb[:, lo : lo + sz])
                    nc.vector.tensor_mul(os_, xs, ta[:, lo : lo + sz])
                else:
                    assert lo == 0 and sz == nsl * FREE
                    cf = nsl * FREE
                    x3 = xbig[:, xoff : xoff + cf].rearrange(
                        "p (j f) -> p j f", j=nsl
                    )
                    u3 = ut[:].rearrange("p (j f) -> p j f", j=nsl)
                    os_ = obig[:, xoff : xoff + cf]
                    o3 = os_.rearrange("p (j f) -> p j f", j=nsl)
                    ta_b = bass.AP(
                        ta[:].tensor, ta[:].offset,
                        [ta[:].ap[0], [0, nsl], ta[:].ap[1]],
                    )
                    tb_b = bass.AP(
                        tb[:].tensor, tb[:].offset,
                        [tb[:].ap[0], [0, nsl], tb[:].ap[1]],
                    )
                    nc.vector.tensor_mul(u3, x3, tb_b)
                    nc.vector.tensor_mul(o3, x3, ta_b)
                    us = ut[:]
                usw = us.rearrange("p (n two) -> p n two", two=2)[:, :, ::-1]
                os3 = os_.rearrange("p (n two) -> p n two", two=2)
                nc.vector.tensor_add(os3, os3, usw)

            row0 = 0
            for ci, nsl in enumerate(CHUNK_PLAN):
                first = ci == 0
                last = ci == len(CHUNK_PLAN) - 1
                cfree = nsl * FREE
                xoff = row0 * FREE
                rows = x_d[row0 * 128 : (row0 + nsl) * 128, :]
                xts = xbig[:, xoff : xoff + cfree]
                if first:
                    # split the first load so compute can start after 0.25 MiB
                    assert nsl == 1
                    h = cfree // 2
                    nc.sync.dma_start(xts[:, :h], rows[:, :h])
                    nc.sync.dma_start(xts[:, h:], rows[:, h:])
                else:
                    src = rows.rearrange("(j p) f -> p j f", j=nsl)
                    nc.sync.dma_start(
                        xts.rearrange("p (j f) -> p j f", j=nsl), src
                    )

                ut = upool.tile([128, cfree], f16, tag="u")
                ots = obig[:, xoff : xoff + cfree]
                orows = o_d[row0 * 128 : (row0 + nsl) * 128, :]

                if first or last:
                    # head chunk in halves: starts computing after the first
                    # half-load (quartering the head measured 72.6 us vs
                    # 72.1 — the first add is table-DMA-gated, not
                    # compute-gated, so smaller pieces don't help). Tail
                    # chunk in quarters: the final store is only 0.125 MB,
                    # shrinking the end-of-run barrier wait for it.
                    nparts = 2 if first else 4
                    h = cfree // nparts
                    for hi in range(nparts):
                        compute(xoff, ut, 1, hi * h, h)
                        if first and hi == 0:
                            # second table halves, after the head's first
                            # half-compute in emission order. Must stay on
                            # the SYNC ring: routing them on the scalar
                            # (store) ring measured 76.4 us (+5) — mixing
                            # load and store directions on one ring
                            # degrades it (same effect as the duplex_dual
                            # microbench).
                            nc.sync.dma_start(tb[:, hf:], tb_d[:, hf:])
                            nc.sync.dma_start(ta[:, hf:], ta_d[:, hf:])
                        nc.scalar.dma_start(
                            orows[:, hi * h : (hi + 1) * h],
                            ots[:, hi * h : (hi + 1) * h],
                        )
                else:
                    compute(xoff, ut, nsl, 0, cfree)
                    dst = orows.rearrange("(j p) f -> p j f", j=nsl)
                    nc.scalar.dma_start(
                        dst, ots.rearrange("p (j f) -> p j f", j=nsl)
                    )
                row0 += nsl

    nc.compile()
    _prog_cache["nc"] = nc
    return nc


def _default_rotate_m(theta=10000.0):
    """Rebuild the reference's rotation buffer if the harness doesn't pass it."""
    half = D // 2
    try:  # replicate the reference's jax-f32 arithmetic exactly if possible
        import jax.numpy as jnp

        pos = np.asarray(jnp.arange(S, dtype=jnp.float32))
        inv_freq = np.asarray(
            theta ** (-(2.0 * jnp.arange(half, dtype=jnp.float32)) / D)
        )
        ang = np.asarray(pos[:, None] * inv_freq[None, :], dtype=np.float32)
        c, s = np.asarray(jnp.cos(ang)), np.asarray(jnp.sin(ang))
    except Exception:
        pos = np.arange(S, dtype=np.float32)
        exp = (-(2.0 * np.arange(half, dtype=np.float32)) / D).astype(np.float32)
        inv_freq = np.power(np.float32(theta), exp, dtype=np.float32)
        ang = (pos[:, None] * inv_freq[None, :]).astype(np.float32)
        c, s = np.cos(ang, dtype=np.float32), np.sin(ang, dtype=np.float32)
    idx = 2 * np.arange(half)
    r = np.zeros((S, D, D), dtype=np.float32)
    r[:, idx, idx] = c
    r[:, idx, idx + 1] = -s
    r[:, idx + 1, idx] = s
    r[:, idx + 1, idx + 1] = c
    return r


def _tables(token_positions, rotate_m):
    """Host-precompute the [128, FREE] fp16 A and B' tables (see docstring)."""
    if rotate_m is None:
        rotate_m = _default_rotate_m()
    r = np.asarray(rotate_m, dtype=np.float32)[np.asarray(token_positions)]
    idx = np.arange(D // 2) * 2
    a = r[:, idx, idx]            # x_even -> out_even
    b = r[:, idx, idx + 1]        # x_odd  -> out_even
    c = r[:, idx + 1, idx + 1]    # x_odd  -> out_odd
    d = r[:, idx + 1, idx]        # x_even -> out_odd
    A = np.empty((S, D), np.float32)
    A[:, 0::2] = a
    A[:, 1::2] = c
    Bp = np.empty((S, D), np.float32)
    Bp[:, 0::2] = d
    Bp[:, 1::2] = b
    return (
        np.ascontiguousarray(A.reshape(128, FREE)).astype(np.float16),
        np.ascontiguousarray(Bp.reshape(128, FREE)).astype(np.float16),
    )


def _in_maps(x, token_positions, rotate_m):
    ta, tb = _tables(token_positions, rotate_m)
    xs = np.asarray(x, dtype=np.float32).astype(np.float16).reshape(
        N_CORES, ROWS, FREE
    )
    xs = np.ascontiguousarray(xs)
    return [{"x": xs[i], "ta": ta, "tb": tb} for i in range(N_CORES)]


def _run(x, token_positions, rotate_m=None, trace=False, trace_cores=None):
    from concourse.bass_utils import run_bass_kernel_spmd

    nc = _build_program()
    in_maps = _in_maps(x, token_positions, rotate_m)
    res = run_bass_kernel_spmd(
        nc,
        in_maps,
        list(range(N_CORES)),
        trace=trace,
        trace_cores=trace_cores,
    )
    out = np.concatenate(
        [res.results[i]["out"].reshape(1, ROWS * FREE) for i in range(N_CORES)]
    ).reshape(B, H, S, D).astype(np.float32)
    return out, res


def kernel(x, token_positions, rotate_m=None, **_unused):
    out, _ = _run(x, token_positions, rotate_m, trace=False)
    return out


# revision 36
# speedup vs baseline: 1.0829x; 1.0262x over previous
"""MetaRoPE kernel for Trainium2, 8 NeuronCores — fp16 I/O + DVE 2x-mode.

Reference computation:
    r = rotate_m[token_positions]            # [S, D, D], block-diag 2x2 rotations
    out = einsum('bhsi,soi->bhso', x, r)     # x: [4, 32, 4096, 64] fp32

Because r is block-diagonal with 2x2 blocks, for each position s and pair k:
    out[2k]   = a*x[2k] + b*x[2k+1]     (a = r[2k,2k],   b = r[2k,2k+1])
    out[2k+1] = c*x[2k+1] + d*x[2k]     (c = r[2k+1,2k+1], d = r[2k+1,2k])
which we compute elementwise as
    out = x * A + pairswap(x * B')
with host-precomputed tables A, B' of shape [S, D]:
    A[s,2k] = a, A[s,2k+1] = c
    B'[s,2k] = d, B'[s,2k+1] = b       (B' is pre-pairswapped so that
                                        pairswap(x*B') lands b*x_odd on even
                                        lanes and d*x_even on odd lanes)

Precision/bandwidth: the correctness gate is rel_err < 2e-2; fp16 end-to-end
(host converts x fp32->fp16, device computes in fp16, host converts the fp16
result back) measures ~1.1e-3 and halves both HBM traffic and DVE element
cost vs fp32. Plain InstTensorTensor ops hit the DVE 2x_1p perf mode with
packed fp16 (~0.5 ns/elem/partition measured, including the stride -1
pair-swap operand). Notes from measurement on HW:
  - scalar_tensor_tensor (fused 3-input op) supports NO DVE perf modes and
    runs ~1.2 ns/elem — slower than two plain ops.
  - 4-dim merged APs (one mul writing u and o via a broadcast x) run ~15%
    slower per element than 3-dim APs.
  - GpSimd tensor ops are Q7 software (~2.5-6.4 ns/elem) AND degrade
    concurrent DVE throughput — never offload to it. Re-verified: a
    single-slab GpSimd sidecar (quarter pieces, interleaved stores)
    measured 81.9 us (+10.6): GpSimd is also the framework's semaphore
    relay hub, so tensor ops in its queue stall every other engine.
  - TENSOR_TENSOR with an int8 output drops to 1x mode (+17 us measured):
    int8 output needs a separate cast, which costs more than it saves here.
  - Stores measure only ~142 GB/s/core (loads ~314 GB/s/core) regardless
    of ring count, burst size or layout; loads and stores overlap fully
    (duplex), but loads split across BOTH rings still cap ~320 GB/s total.
    GpSimd SWDGE stores hit the same wall (~130 GB/s, desc-gen overhead)
    and scalar+gpsimd mixed stores do NOT combine — the write cap is
    global across all three DMA paths.
  - int8 stores with the cast on ScalarE (tables pre-scaled 127/8):
    77.0 us; with the last chunk's adds writing int8 directly (1x) to
    skip the final cast hop: 76.1 us. Root cause int8 cannot win: at
    the 36-op plan the fp16 store drain (ends ~68.5) already finishes
    inside the add-stream shadow (~68-69) — the store wall is NOT
    binding, so halving store bytes saves ~1 us of final drain while
    any cast path costs at least that in added stream/queue latency.
  - Chunk-plan space is closed from both ends: early chunks must be
    small for LOAD pacing ([1,2,4,4,4,1] = 88.4 us, 4-slab chunks
    before ~row 8 starve) and late chunks small for STORE pacing
    ([1,1,2,4,6,1,1] = 76.7 us — a 6-slab chunk's 3 MB store cannot
    issue until its single add completes, pushing the drain past the
    stream end). [1,1,2,4,4,2,1,1] is the optimum.
  - The ~2 us early TT gap (second compute piece waiting second table
    halves) is load-bandwidth-conserved: 1.5 MB of tables+slab0 must
    land before second-half compute vs ~0.7 us of available compute —
    no reorder removes it; only smaller tables would, and fp8 tables
    fail the accuracy gate.
  - Chunk plans feeding 4-slab chunks before ~row 8 starve the DVE (the
    load stream delivers ~1 slab/1.64 us after tables): [1,2,4,4,4,1]
    measured 88.4 us. The shipped plan keeps 1/1/2-slab chunks up front.
  - Run-to-run variance for the same NEFF is +-1.5-2 us (72.07-74.26
    observed), from engine-init and DMA-latency jitter.
  - The idle TensorE cannot profitably offload elementwise work: matmul
    output is PSUM-fp32-only on TRN2, and evacuating PSUM costs ~1.3
    ns/elem (ScalarE flat ACTIVATE copy; 4.9 ns/elem if the copy AP is
    strided) per pass — more than the ~1.6 ns/elem the DVE 3-pass spends
    outright. A full two-round block-diag-matmul hybrid (R_s = R_{64q} R_r,
    96 weight tiles, 14 DVE + 2 PE slabs) was built and measured 75-119 us
    across schedules: the extra ~4 MB of weight/relayout loads starves the
    DVE mid-stream at the shared load ceiling (see kernel_hybrid_pe.py.bak).

Sharding: x reshaped to [128 (b,h) slabs, 4096, 64]; 16 slabs per core.
Each slab [4096*64] is viewed as [128 partitions, 2048 free] (contiguous per
partition; partition p holds positions 32p..32p+31). Tables are replicated
to every core as [128, 2048] fp16 tiles that match that layout for every
slab.

Per core the 16 slabs are processed in chunks (CHUNK_PLAN, tapered small at
the ends to shrink pipeline ramp/tail; 4-slab middle chunks cut the DVE
instruction count ~42 -> ~30, saving ~1.6 us vs the all-2-slab plan). Each
chunk: one load (HWDGE on the sync ring), two DVE tensor_muls (tables
broadcast across the chunk's slabs via a step-0 AP dim) + one pair-swapped
in-place tensor_add, one store (HWDGE on the scalar ring). Table halves
split across rings: first halves on the scalar ring up front, second halves
on the sync ring emitted after the head chunk's first half-compute, so that
compute (which needs only table cols [0:1024)) starts ~3.5 us earlier —
tile deps follow emission order, and every DVE op emitted after a DMA to
tb/ta waits on it.

Measured: 71.2-72.5 us HW exec across runs (149.8 us fp32 naive ->
74.3 us all-2-slab plan -> this). Closed-form breakdown: first TT at
~10.8-11.4 (engine init ~7 + table/first-chunk DMA), one conserved
~2 us gap (second table halves), 54.4 us TT stream (51.2 roofline +
~3 instr overhead, otherwise gapless), last store ~1.5, end barrier
~2.7. Stores drain continuously from ~9.5 and finish inside the
stream shadow. Every component is individually floored: init/barrier
are framework-fixed, the stream is at the 7-lane-crossbar silicon
limit for fp16 TT, the early gap is load-BW-conserved, and the
int8 / TensorE / GpSimd alternatives all measure worse (notes above).
"""

import sys

import numpy as np

_TRN_REPO = "/opt/trn_rl_repo"
if _TRN_REPO not in sys.path:
    sys.path.insert(0, _TRN_REPO)

B, H, S, D = 4, 32, 4096, 64
BH = B * H                      # 128 (b,h) slabs
N_CORES = 8
BH_PER_CORE = BH // N_CORES     # 16 slabs per core
S_PER_CORE = S // N_CORES       # 512 positions per core (S-sharding)
FREE = (S_PER_CORE // 128) * D  # 256 free elements per partition per slab
SLABS = BH                      # 128 slabs per core (one per (b,h))
# slabs per chunk, tapered: small first chunks for LOAD pacing, small
# last chunks so late stores can issue early (STORE pacing)
CHUNK_PLAN = [4, 4, 8, 16, 16, 16, 16, 16, 16, 8, 4, 2, 1, 1]
assert sum(CHUNK_PLAN) == SLABS
U_BUFS = 2

_prog_cache = {}


def _build_program():
    """Build (and cache) the SPMD Bass program for one core."""
    if "nc" in _prog_cache:
        return _prog_cache["nc"]

    import concourse.bacc as bacc
    import concourse.bass as bass
    import concourse.mybir as mybir
    import concourse.tile as tile

    f16 = mybir.dt.float16
    nc = bacc.Bacc(
        "TRN2", target_bir_lowering=False, debug=False, num_devices=N_CORES
    )
    x_d = nc.dram_tensor("x", [128, SLABS * FREE], f16, kind="ExternalInput").ap()
    ta_d = nc.dram_tensor("ta", [128, FREE], f16, kind="ExternalInput").ap()
    tb_d = nc.dram_tensor("tb", [128, FREE], f16, kind="ExternalInput").ap()
    o_d = nc.dram_tensor("out", [128, SLABS * FREE], i8 if False else f16, kind="ExternalOutput").ap()

    with tile.TileContext(nc) as tc:
        with (
            tc.tile_pool(name="tabs", bufs=1) as tabs,
            tc.tile_pool(name="xbig", bufs=1) as xbigp,
            tc.tile_pool(name="u", bufs=U_BUFS) as upool,
            tc.tile_pool(name="obig", bufs=1) as obigp,
        ):
            # With S-sharding the per-core tables are only [128, 256] fp16
            # (64 KB each): both load whole on the scalar ring and land
            # before the first x chunk — no half-table machinery, no
            # table-gated gap in the DVE stream.
            tb = tabs.tile([128, FREE], f16)
            ta = tabs.tile([128, FREE], f16)
            nc.scalar.dma_start(tb[:], tb_d[:])
            nc.scalar.dma_start(ta[:], ta_d[:])

            xbig = xbigp.tile([128, SLABS * FREE], f16)
            obig = obigp.tile([128, SLABS * FREE], f16)

            def compute(xoff, ut, nsl):
                """u = x*tb; o = x*ta; o += pairswap(u) for nsl slabs at
                element offset xoff (tables broadcast across slabs via a
                step-0 AP dim)."""
                cf = nsl * FREE
                x3 = xbig[:, xoff : xoff + cf].rearrange(
                    "p (j f) -> p j f", j=nsl
                )
                u3 = ut[:, :cf].rearrange("p (j f) -> p j f", j=nsl)
                os_ = obig[:, xoff : xoff + cf]
                o3 = os_.rearrange("p (j f) -> p j f", j=nsl)
                ta_b = bass.AP(
                    ta[:].tensor, ta[:].offset,
                    [ta[:].ap[0], [0, nsl], ta[:].ap[1]],
                )
                tb_b = bass.AP(
                    tb[:].tensor, tb[:].offset,
                    [tb[:].ap[0], [0, nsl], tb[:].ap[1]],
                )
                nc.vector.tensor_mul(u3, x3, tb_b)
                nc.vector.tensor_mul(o3, x3, ta_b)
                us = ut[:, :cf]
                usw = us.rearrange("p (n two) -> p n two", two=2)[:, :, ::-1]
                os3 = os_.rearrange("p (n two) -> p n two", two=2)
                nc.vector.tensor_add(os3, os3, usw)

            row0 = 0
            for ci, nsl in enumerate(CHUNK_PLAN):
                cfree = nsl * FREE
                xoff = row0 * FREE
                nc.sync.dma_start(
                    xbig[:, xoff : xoff + cfree], x_d[:, xoff : xoff + cfree]
                )
                ut = upool.tile([128, cfree], f16, tag="u")
                compute(xoff, ut, nsl)
                nc.scalar.dma_start(
                    o_d[:, xoff : xoff + cfree], obig[:, xoff : xoff + cfree]
                )
                row0 += nsl

    nc.compile()
    _prog_cache["nc"] = nc
    return nc


def _default_rotate_m(theta=10000.0):
    """Rebuild the reference's rotation buffer if the harness doesn't pass it."""
    half = D // 2
    try:  # replicate the reference's jax-f32 arithmetic exactly if possible
        import jax.numpy as jnp

        pos = np.asarray(jnp.arange(S, dtype=jnp.float32))
        inv_freq = np.asarray(
            theta ** (-(2.0 * jnp.arange(half, dtype=jnp.float32)) / D)
        )
        ang = np.asarray(pos[:, None] * inv_freq[None, :], dtype=np.float32)
        c, s = np.asarray(jnp.cos(ang)), np.asarray(jnp.sin(ang))
    except Exception:
        pos = np.arange(S, dtype=np.float32)
        exp = (-(2.0 * np.arange(half, dtype=np.float32)) / D).astype(np.float32)
        inv_freq = np.power(np.float32(theta), exp, dtype=np.float32)
        ang = (pos[:, None] * inv_freq[None, :]).astype(np.float32)
        c, s = np.cos(ang, dtype=np.float32), np.sin(ang, dtype=np.float32)
    idx = 2 * np.arange(half)
    r = np.zeros((S, D, D), dtype=np.float32)
    r[:, idx, idx] = c
    r[:, idx, idx + 1] = -s
    r[:, idx + 1, idx] = s
    r[:, idx + 1, idx + 1] = c
    return r


def _tables(token_positions, rotate_m):
    """Host-precompute per-core [128, FREE] fp16 A and B' tables for the
    core's 512 positions (partition p holds positions 4p..4p+3)."""
    if rotate_m is None:
        rotate_m = _default_rotate_m()
    r = np.asarray(rotate_m, dtype=np.float32)[np.asarray(token_positions)]
    idx = np.arange(D // 2) * 2
    a = r[:, idx, idx]
    b = r[:, idx, idx + 1]
    c = r[:, idx + 1, idx + 1]
    d = r[:, idx + 1, idx]
    A = np.empty((S, D), np.float32)
    A[:, 0::2] = a
    A[:, 1::2] = c
    Bp = np.empty((S, D), np.float32)
    Bp[:, 0::2] = d
    Bp[:, 1::2] = b
    A8 = A.reshape(N_CORES, 128, FREE).astype(np.float16)
    B8 = Bp.reshape(N_CORES, 128, FREE).astype(np.float16)
    return A8, B8


def _in_maps(x, token_positions, rotate_m):
    A8, B8 = _tables(token_positions, rotate_m)
    xs = np.asarray(x, dtype=np.float32).astype(np.float16).reshape(
        BH, N_CORES, 128, 4 * D
    )
    # x_d[core][p, k*FREE + f] = x[bh=k, s=512*core + 4p + s4, d]
    xt = np.ascontiguousarray(xs.transpose(1, 2, 0, 3))  # [core, p, bh, 256]
    return [
        {
            "x": xt[i].reshape(128, SLABS * FREE),
            "ta": np.ascontiguousarray(A8[i]),
            "tb": np.ascontiguousarray(B8[i]),
        }
        for i in range(N_CORES)
    ]


def _run(x, token_positions, rotate_m=None, trace=False, trace_cores=None):
    from concourse.bass_utils import run_bass_kernel_spmd

    nc = _build_program()
    in_maps = _in_maps(x, token_positions, rotate_m)
    res = run_bass_kernel_spmd(
        nc,
        in_maps,
        list(range(N_CORES)),
        trace=trace,
        trace_cores=trace_cores,
    )
    outs = np.stack(
        [res.results[i]["out"].reshape(128, BH, FREE) for i in range(N_CORES)]
    )  # [core, p, bh, 256]
    out = np.ascontiguousarray(outs.transpose(2, 0, 1, 3)).reshape(
        BH, S, D
    ).reshape(B, H, S, D).astype(np.float32)
    return out, res


def kernel(x, token_positions, rotate_m=None, **_unused):
    out, _ = _run(x, token_positions, rotate_m, trace=False)
    return out
